# revision 4
# baseline (speedup 1.0000x reference)
"""Trainium2 Bass kernel for a 3-layer GCN encoder (B=32, N=1000, D=256).

Math: the reference's normalized adjacency for a fully-connected graph
(self_loop=False -> adj = ones) is A_norm = ones(N,N)/N, so the
"aggregation" einsum is a mean over nodes broadcast back to every node.
Since mean o linear = linear o mean and the mean is idempotent across
layers (h constant over nodes after layer 0), the whole network collapses
to, per batch b:

    m_b  = mean_n node_feature[b, n, :]          # (D,)
    h1_b = relu(m_b @ W0 + b0)
    h2_b = relu(h1_b @ W1 + b1)
    h3_b = h2_b @ W2 + b2
    out[b, n, :] = node_feature[b, n, :] + h3_b  # broadcast residual

Sharding: data-parallel over batch, 4 batches per core on 8 cores.
Each core streams its 4 MB shard in, computes per-batch column sums with
the PE (ones-vector matmuls), runs the tiny 256x256 chain in transposed
orientation (bias+relu fused into ScalarE activations), broadcasts h3
across partitions with a rank-1 matmul, adds in place on VectorE, and
streams the result back out.
"""

import numpy as np

import concourse.bacc as bacc
import concourse.bass as bass
import concourse.mybir as mybir
import concourse.tile as tile
from concourse.bass_utils import run_bass_kernel_spmd

F32 = mybir.dt.float32

B, N, D, L = 32, 1000, 256, 3
NCORES = 8
NB = B // NCORES  # batches per core
P = 125           # partition rows per node-slice
T = N // P        # node-slices per batch
HALF = 128        # half of D (partition dim for transposed chain)

_NC_CACHE = {}


def _build_nc():
    nc = bacc.Bacc("TRN2", target_bir_lowering=False, debug=False)

    nf_d = nc.dram_tensor("nf", [NB, N, D], F32, kind="ExternalInput")
    w_d = nc.dram_tensor("w", [L, D, D], F32, kind="ExternalInput")
    bvec_d = nc.dram_tensor("bvec", [HALF, 2 * L], F32, kind="ExternalInput")
    out_d = nc.dram_tensor("out", [NB, N, D], F32, kind="ExternalOutput")

    ones_col_d = nc.inline_tensor(np.ones((P, 1), np.float32), "ones_col")
    ones_row_d = nc.inline_tensor(np.ones((1, P), np.float32), "ones_row")
    ident_d = nc.inline_tensor(np.eye(HALF, dtype=np.float32), "ident")

    relu = mybir.ActivationFunctionType.Relu
    ident_fn = mybir.ActivationFunctionType.Identity
    copy_fn = mybir.ActivationFunctionType.Copy

    with tile.TileContext(nc) as tc:
        with (
            tc.tile_pool(name="const", bufs=1) as cpool,
            tc.tile_pool(name="data", bufs=NB) as dpool,
            tc.tile_pool(name="vec", bufs=8) as vpool,
            tc.tile_pool(name="bcast", bufs=2) as bpool,
            tc.tile_pool(name="ps_sum", bufs=2, space=bass.MemorySpace.PSUM) as ps_sum,
            tc.tile_pool(name="ps_chain", bufs=2, space=bass.MemorySpace.PSUM) as ps_chain,
            tc.tile_pool(name="ps_row", bufs=2, space=bass.MemorySpace.PSUM) as ps_row,
            tc.tile_pool(name="ps_bc", bufs=2, space=bass.MemorySpace.PSUM) as ps_bc,
        ):
            # ---- constants ----
            w_sb = []
            for l in range(L):
                wt = cpool.tile([HALF, 2, D], F32, tag=f"w{l}", name=f"w{l}")
                nc.sync.dma_start(wt[:], w_d[l].rearrange("(kc k) e -> k kc e", k=HALF))
                w_sb.append(wt)
            bvec = cpool.tile([HALF, 2 * L], F32, tag="bvec", name="bvec")
            nc.sync.dma_start(bvec[:], bvec_d[:])
            ones_col = cpool.tile([P, 1], F32, tag="ones_col", name="ones_col")
            nc.sync.dma_start(ones_col[:], ones_col_d[:])
            ones_row = cpool.tile([1, P], F32, tag="ones_row", name="ones_row")
            nc.sync.dma_start(ones_row[:], ones_row_d[:])
            ident = cpool.tile([HALF, HALF], F32, tag="ident", name="ident")
            nc.sync.dma_start(ident[:], ident_d[:])

            for b in range(NB):
                nf_t = dpool.tile([P, T, D], F32, tag="nf", name=f"nf{b}")
                nc.sync.dma_start(nf_t[:], nf_d[b].rearrange("(t p) d -> p t d", p=P))

                # ---- per-batch column sums, transposed orientation ----
                # sumT[half][d, 0] = sum over all N rows of nf[:, half*128+d]
                h = []
                for mh in range(2):
                    ps = ps_sum.tile([HALF, 1], F32, tag="ps_s", name=f"ps_s{b}_{mh}")
                    for t in range(T):
                        nc.tensor.matmul(
                            ps[:],
                            nf_t[:, t, mh * HALF:(mh + 1) * HALF],
                            ones_col[:],
                            start=(t == 0),
                            stop=(t == T - 1),
                        )
                    s = vpool.tile([HALF, 1], F32, tag="hT", name=f"sum{b}_{mh}")
                    # mean = sum / N, fused into the PSUM->SBUF copy
                    nc.scalar.activation(s[:], ps[:], copy_fn, scale=1.0 / N)
                    h.append(s)

                # ---- 3-layer chain in transposed orientation ----
                for l in range(L):
                    hn = []
                    for mh in range(2):
                        pc = ps_chain.tile([HALF, 1], F32, tag="ps_c", name=f"ps_c{b}_{l}_{mh}")
                        for kc in range(2):
                            nc.tensor.matmul(
                                pc[:],
                                w_sb[l][:, kc, mh * HALF:(mh + 1) * HALF],
                                h[kc][:],
                                start=(kc == 0),
                                stop=(kc == 1),
                            )
                        ht = vpool.tile([HALF, 1], F32, tag="hT", name=f"h{b}_{l}_{mh}")
                        nc.scalar.activation(
                            ht[:],
                            pc[:],
                            relu if l < L - 1 else ident_fn,
                            bias=bvec[:, 2 * l + mh:2 * l + mh + 1],
                        )
                        hn.append(ht)
                    h = hn

                # ---- transpose h3 back to a row, broadcast across partitions ----
                pr = ps_row.tile([1, D], F32, tag="ps_r", name=f"ps_r{b}")
                for kc in range(2):
                    nc.tensor.transpose(
                        pr[0:1, kc * HALF:(kc + 1) * HALF], h[kc][:], ident[:]
                    )
                h3row = vpool.tile([1, D], F32, tag="h3row", name=f"h3row{b}")
                nc.scalar.activation(h3row[:], pr[:], copy_fn)

                pb = ps_bc.tile([P, D], F32, tag="ps_b", name=f"ps_b{b}")
                nc.tensor.matmul(pb[:], ones_row[:], h3row[:], start=True, stop=True)
                bc = bpool.tile([P, D], F32, tag="bc", name=f"bc{b}")
                nc.vector.tensor_copy(bc[:], pb[:])

                # ---- residual add in place, then stream out ----
                for t in range(T):
                    nc.vector.tensor_add(nf_t[:, t, :], nf_t[:, t, :], bc[:])
                nc.sync.dma_start(out_d[b].rearrange("(t p) d -> p t d", p=P), nf_t[:])

    nc.compile()
    return nc


def _get_nc():
    if "nc" not in _NC_CACHE:
        _NC_CACHE["nc"] = _build_nc()
    return _NC_CACHE["nc"]


def _make_in_maps(node_feature, Ws, bs):
    nf = np.ascontiguousarray(np.asarray(node_feature, dtype=np.float32))
    w = np.ascontiguousarray(np.asarray(Ws, dtype=np.float32))
    b = np.asarray(bs, dtype=np.float32)
    # bvec[p, 2*l + half] = bs[l, half*128 + p]
    bvec = np.ascontiguousarray(
        b.reshape(L, 2, HALF).transpose(2, 0, 1).reshape(HALF, 2 * L)
    )
    in_maps = []
    for i in range(NCORES):
        in_maps.append(
            {
                "nf": np.ascontiguousarray(nf[i * NB:(i + 1) * NB]),
                "w": w,
                "bvec": bvec,
            }
        )
    return in_maps


def run_on_hw(node_feature, Ws, bs):
    nc = _get_nc()
    res = run_bass_kernel_spmd(
        nc,
        _make_in_maps(node_feature, Ws, bs),
        list(range(NCORES)),
        trace=False,
    )
    out = np.concatenate([res.results[i]["out"] for i in range(NCORES)], axis=0)
    return out, res


def kernel(x, node_feature, Ws, bs):
    node_feature = np.asarray(node_feature, dtype=np.float32)
    out, _ = run_on_hw(node_feature, Ws, bs)
    return out, node_feature


# ---------------------------------------------------------------------------
# Timing runner: same PJRT path as run_bass_kernel_spmd under axon, but with
# the jitted executable cached so repeated executions can be timed without
# re-tracing/re-compiling. Used by test.py only.
# ---------------------------------------------------------------------------


class _Runner:
    def __init__(self):
        import jax
        from jax.experimental.shard_map import shard_map
        from jax.sharding import Mesh, NamedSharding, PartitionSpec

        from concourse.bass2jax import (
            _bass_exec_p,
            install_neuronx_cc_hook,
            partition_id_tensor,
        )

        install_neuronx_cc_hook()
        self.jax = jax
        nc = _get_nc()
        partition_name = (
            nc.partition_id_tensor.name if nc.partition_id_tensor else None
        )
        in_names, out_names, out_avals, zero_outs = [], [], [], []
        for alloc in nc.m.functions[0].allocations:
            if not isinstance(alloc, mybir.MemoryLocationSet):
                continue
            name = alloc.memorylocations[0].name
            if alloc.kind == "ExternalInput":
                if name != partition_name:
                    in_names.append(name)
            elif alloc.kind == "ExternalOutput":
                shape = tuple(alloc.tensor_shape)
                dt = mybir.dt.np(alloc.dtype)
                out_names.append(name)
                out_avals.append(jax.core.ShapedArray(shape, dt))
                zero_outs.append(np.zeros(shape, dt))
        self.in_names = in_names
        self.out_names = out_names
        self.out_avals = out_avals
        self.zero_outs = zero_outs
        n_params, n_outs = len(in_names), len(out_names)
        all_names = tuple(
            in_names + out_names + ([partition_name] if partition_name else [])
        )

        def _body(*args):
            operands = list(args)
            if partition_name is not None:
                operands.append(partition_id_tensor())
            outs = _bass_exec_p.bind(
                *operands,
                out_avals=tuple(out_avals),
                in_names=all_names,
                out_names=tuple(out_names),
                lowering_input_output_aliases=(),
                sim_require_finite=True,
                sim_require_nnan=True,
                nc=nc,
            )
            return tuple(outs)

        devices = jax.devices()[:NCORES]
        self.mesh = Mesh(np.asarray(devices), ("core",))
        self.sharding = NamedSharding(self.mesh, PartitionSpec("core"))
        in_specs = (PartitionSpec("core"),) * (n_params + n_outs)
        out_specs = (PartitionSpec("core"),) * n_outs
        self.jitted = jax.jit(
            shard_map(
                _body,
                mesh=self.mesh,
                in_specs=in_specs,
                out_specs=out_specs,
                check_rep=False,
            ),
            donate_argnums=tuple(range(n_params, n_params + n_outs)),
            keep_unused=True,
        )

    def stage_inputs(self, in_maps):
        concat = [
            np.concatenate([m[name] for m in in_maps], axis=0)
            for name in self.in_names
        ]
        return [self.jax.device_put(a, self.sharding) for a in concat]

    def stage_zeros(self):
        return [
            self.jax.device_put(
                np.zeros((NCORES * z.shape[0], *z.shape[1:]), z.dtype), self.sharding
            )
            for z in self.zero_outs
        ]

    def run(self, dev_inputs, dev_zeros):
        return self.jitted(*dev_inputs, *dev_zeros)


_RUNNER_CACHE = {}


def get_runner():
    if "r" not in _RUNNER_CACHE:
        _RUNNER_CACHE["r"] = _Runner()
    return _RUNNER_CACHE["r"]


# revision 10
# speedup vs baseline: 49.5688x; 49.5688x over previous
"""Trainium2 Bass kernel for a 3-layer GCN encoder (B=32, N=1000, D=256).

Math: the reference's normalized adjacency for a fully-connected graph
(self_loop=False -> adj = ones) is A_norm = ones(N,N)/N, so the
"aggregation" einsum is a mean over nodes broadcast back to every node.
Since mean o linear = linear o mean and the mean is idempotent across
layers (h constant over nodes after layer 0), the whole network collapses
to, per batch b:

    m_b  = mean_n node_feature[b, n, :]          # (D,)
    h1_b = relu(m_b @ W0 + b0)
    h2_b = relu(h1_b @ W1 + b1)
    h3_b = h2_b @ W2 + b2
    out[b, n, :] = node_feature[b, n, :] + h3_b  # broadcast residual

Sharding: data-parallel over batch, 4 batches per core on 8 cores.
Each core streams its 4 MB shard in, computes per-batch column sums with
the PE (ones-vector matmuls), runs the tiny 256x256 chain in transposed
orientation (bias+relu fused into ScalarE activations), broadcasts h3
across partitions with a rank-1 matmul, adds in place on VectorE, and
streams the result back out.
"""

import numpy as np

import concourse.bacc as bacc
import concourse.bass as bass
import concourse.mybir as mybir
import concourse.tile as tile
from concourse.bass_utils import run_bass_kernel_spmd

F32 = mybir.dt.float32

B, N, D, L = 32, 1000, 256, 3
NCORES = 8
NB = B // NCORES  # batches per core
P = 125           # partition rows per node-slice
T = N // P        # node-slices per batch
HALF = 128        # half of D (partition dim for transposed chain)

_NC_CACHE = {}


def _build_nc(reps=1):
    nc = bacc.Bacc("TRN2", target_bir_lowering=False, debug=False)

    nf_d = nc.dram_tensor("nf", [NB, N, D], F32, kind="ExternalInput")
    w_d = nc.dram_tensor("w", [L, D, D], F32, kind="ExternalInput")
    bvec_d = nc.dram_tensor("bvec", [HALF, 2 * L], F32, kind="ExternalInput")
    out_d = nc.dram_tensor("out", [NB, N, D], F32, kind="ExternalOutput")

    ones_col_d = nc.inline_tensor(np.ones((P, 1), np.float32), "ones_col")
    ones_row_d = nc.inline_tensor(np.ones((1, P), np.float32), "ones_row")
    ident_d = nc.inline_tensor(np.eye(HALF, dtype=np.float32), "ident")

    relu = mybir.ActivationFunctionType.Relu
    ident_fn = mybir.ActivationFunctionType.Identity
    copy_fn = mybir.ActivationFunctionType.Copy

    with tile.TileContext(nc) as tc:
        with (
            tc.tile_pool(name="const", bufs=1) as cpool,
            tc.tile_pool(name="data", bufs=NB) as dpool,
            tc.tile_pool(name="vec", bufs=8) as vpool,
            tc.tile_pool(name="bcast", bufs=2) as bpool,
            tc.tile_pool(name="ps_sum", bufs=2, space=bass.MemorySpace.PSUM) as ps_sum,
            tc.tile_pool(name="ps_chain", bufs=2, space=bass.MemorySpace.PSUM) as ps_chain,
            tc.tile_pool(name="ps_row", bufs=2, space=bass.MemorySpace.PSUM) as ps_row,
            tc.tile_pool(name="ps_bc", bufs=2, space=bass.MemorySpace.PSUM) as ps_bc,
        ):
            # ---- constants ----
            w_sb = []
            for l in range(L):
                wt = cpool.tile([HALF, 2, D], F32, tag=f"w{l}", name=f"w{l}")
                nc.sync.dma_start(wt[:], w_d[l].rearrange("(kc k) e -> k kc e", k=HALF))
                w_sb.append(wt)
            bvec = cpool.tile([HALF, 2 * L], F32, tag="bvec", name="bvec")
            nc.sync.dma_start(bvec[:], bvec_d[:])
            ones_col = cpool.tile([P, 1], F32, tag="ones_col", name="ones_col")
            nc.sync.dma_start(ones_col[:], ones_col_d[:])
            ones_row = cpool.tile([1, P], F32, tag="ones_row", name="ones_row")
            nc.sync.dma_start(ones_row[:], ones_row_d[:])
            ident = cpool.tile([HALF, HALF], F32, tag="ident", name="ident")
            nc.sync.dma_start(ident[:], ident_d[:])

            def batch_body():
              for b in range(NB):
                nf_t = dpool.tile([P, T, D], F32, tag="nf", name=f"nf{b}")
                nc.sync.dma_start(nf_t[:], nf_d[b].rearrange("(t p) d -> p t d", p=P))

                # ---- per-batch column sums, transposed orientation ----
                # sumT[half][d, 0] = sum over all N rows of nf[:, half*128+d]
                h = []
                for mh in range(2):
                    ps = ps_sum.tile([HALF, 1], F32, tag="ps_s", name=f"ps_s{b}_{mh}")
                    for t in range(T):
                        nc.tensor.matmul(
                            ps[:],
                            nf_t[:, t, mh * HALF:(mh + 1) * HALF],
                            ones_col[:],
                            start=(t == 0),
                            stop=(t == T - 1),
                        )
                    s = vpool.tile([HALF, 1], F32, tag="hT", name=f"sum{b}_{mh}")
                    # mean = sum / N, fused into the PSUM->SBUF copy
                    nc.scalar.activation(s[:], ps[:], copy_fn, scale=1.0 / N)
                    h.append(s)

                # ---- 3-layer chain in transposed orientation ----
                for l in range(L):
                    hn = []
                    for mh in range(2):
                        pc = ps_chain.tile([HALF, 1], F32, tag="ps_c", name=f"ps_c{b}_{l}_{mh}")
                        for kc in range(2):
                            nc.tensor.matmul(
                                pc[:],
                                w_sb[l][:, kc, mh * HALF:(mh + 1) * HALF],
                                h[kc][:],
                                start=(kc == 0),
                                stop=(kc == 1),
                            )
                        ht = vpool.tile([HALF, 1], F32, tag="hT", name=f"h{b}_{l}_{mh}")
                        nc.scalar.activation(
                            ht[:],
                            pc[:],
                            relu if l < L - 1 else ident_fn,
                            bias=bvec[:, 2 * l + mh:2 * l + mh + 1],
                        )
                        hn.append(ht)
                    h = hn

                # ---- transpose h3 back to a row, broadcast across partitions ----
                pr = ps_row.tile([1, D], F32, tag="ps_r", name=f"ps_r{b}")
                for kc in range(2):
                    nc.tensor.transpose(
                        pr[0:1, kc * HALF:(kc + 1) * HALF], h[kc][:], ident[:]
                    )
                h3row = vpool.tile([1, D], F32, tag="h3row", name=f"h3row{b}")
                nc.scalar.activation(h3row[:], pr[:], copy_fn)

                pb = ps_bc.tile([P, D], F32, tag="ps_b", name=f"ps_b{b}")
                nc.tensor.matmul(pb[:], ones_row[:], h3row[:], start=True, stop=True)
                bc = bpool.tile([P, D], F32, tag="bc", name=f"bc{b}")
                nc.vector.tensor_copy(bc[:], pb[:])

                # ---- residual add in place, then stream out ----
                for t in range(T):
                    nc.vector.tensor_add(nf_t[:, t, :], nf_t[:, t, :], bc[:])
                nc.sync.dma_start(out_d[b].rearrange("(t p) d -> p t d", p=P), nf_t[:])

            if reps == 1:
                batch_body()
            else:
                with tc.For_i(0, reps, 1):
                    batch_body()

    nc.compile()
    return nc


def _get_nc(reps=1):
    if reps not in _NC_CACHE:
        _NC_CACHE[reps] = _build_nc(reps)
    return _NC_CACHE[reps]


def _make_in_maps(node_feature, Ws, bs):
    nf = np.ascontiguousarray(np.asarray(node_feature, dtype=np.float32))
    w = np.ascontiguousarray(np.asarray(Ws, dtype=np.float32))
    b = np.asarray(bs, dtype=np.float32)
    # bvec[p, 2*l + half] = bs[l, half*128 + p]
    bvec = np.ascontiguousarray(
        b.reshape(L, 2, HALF).transpose(2, 0, 1).reshape(HALF, 2 * L)
    )
    in_maps = []
    for i in range(NCORES):
        in_maps.append(
            {
                "nf": np.ascontiguousarray(nf[i * NB:(i + 1) * NB]),
                "w": w,
                "bvec": bvec,
            }
        )
    return in_maps


def run_on_hw(node_feature, Ws, bs):
    nc = _get_nc()
    res = run_bass_kernel_spmd(
        nc,
        _make_in_maps(node_feature, Ws, bs),
        list(range(NCORES)),
        trace=False,
    )
    out = np.concatenate([res.results[i]["out"] for i in range(NCORES)], axis=0)
    return out, res


def kernel(x, node_feature, Ws, bs):
    node_feature = np.asarray(node_feature, dtype=np.float32)
    out, _ = run_on_hw(node_feature, Ws, bs)
    return out, node_feature


# ---------------------------------------------------------------------------
# Timing runner: same PJRT path as run_bass_kernel_spmd under axon, but with
# the jitted executable cached so repeated executions can be timed without
# re-tracing/re-compiling. Used by test.py only.
# ---------------------------------------------------------------------------


class _Runner:
    def __init__(self, reps=1):
        import jax
        from jax.experimental.shard_map import shard_map
        from jax.sharding import Mesh, NamedSharding, PartitionSpec

        from concourse.bass2jax import (
            _bass_exec_p,
            install_neuronx_cc_hook,
            partition_id_tensor,
        )

        install_neuronx_cc_hook()
        self.jax = jax
        nc = _get_nc(reps)
        partition_name = (
            nc.partition_id_tensor.name if nc.partition_id_tensor else None
        )
        in_names, out_names, out_avals, zero_outs = [], [], [], []
        for alloc in nc.m.functions[0].allocations:
            if not isinstance(alloc, mybir.MemoryLocationSet):
                continue
            name = alloc.memorylocations[0].name
            if alloc.kind == "ExternalInput":
                if name != partition_name:
                    in_names.append(name)
            elif alloc.kind == "ExternalOutput":
                shape = tuple(alloc.tensor_shape)
                dt = mybir.dt.np(alloc.dtype)
                out_names.append(name)
                out_avals.append(jax.core.ShapedArray(shape, dt))
                zero_outs.append(np.zeros(shape, dt))
        self.in_names = in_names
        self.out_names = out_names
        self.out_avals = out_avals
        self.zero_outs = zero_outs
        n_params, n_outs = len(in_names), len(out_names)
        all_names = tuple(
            in_names + out_names + ([partition_name] if partition_name else [])
        )

        def _body(*args):
            operands = list(args)
            if partition_name is not None:
                operands.append(partition_id_tensor())
            outs = _bass_exec_p.bind(
                *operands,
                out_avals=tuple(out_avals),
                in_names=all_names,
                out_names=tuple(out_names),
                lowering_input_output_aliases=(),
                sim_require_finite=True,
                sim_require_nnan=True,
                nc=nc,
            )
            return tuple(outs)

        devices = jax.devices()[:NCORES]
        self.mesh = Mesh(np.asarray(devices), ("core",))
        self.sharding = NamedSharding(self.mesh, PartitionSpec("core"))
        in_specs = (PartitionSpec("core"),) * (n_params + n_outs)
        out_specs = (PartitionSpec("core"),) * n_outs
        self.jitted = jax.jit(
            shard_map(
                _body,
                mesh=self.mesh,
                in_specs=in_specs,
                out_specs=out_specs,
                check_rep=False,
            ),
            donate_argnums=tuple(range(n_params, n_params + n_outs)),
            keep_unused=True,
        )

    def stage_inputs(self, in_maps):
        concat = [
            np.concatenate([m[name] for m in in_maps], axis=0)
            for name in self.in_names
        ]
        return [self.jax.device_put(a, self.sharding) for a in concat]

    def stage_zeros(self):
        return [
            self.jax.device_put(
                np.zeros((NCORES * z.shape[0], *z.shape[1:]), z.dtype), self.sharding
            )
            for z in self.zero_outs
        ]

    def run(self, dev_inputs, dev_zeros):
        return self.jitted(*dev_inputs, *dev_zeros)


_RUNNER_CACHE = {}


def get_runner(reps=1):
    if reps not in _RUNNER_CACHE:
        _RUNNER_CACHE[reps] = _Runner(reps)
    return _RUNNER_CACHE[reps]


# revision 15
# speedup vs baseline: 50.6348x; 1.0215x over previous
"""Trainium2 Bass kernel for a 3-layer GCN encoder (B=32, N=1000, D=256).

Math: the reference's normalized adjacency for a fully-connected graph
(self_loop=False -> adj = ones) is A_norm = ones(N,N)/N, so the
"aggregation" einsum is a mean over nodes broadcast back to every node.
Since mean o linear = linear o mean and the mean is idempotent across
layers (h is constant over nodes after layer 0), the whole network
collapses to, per batch b:

    m_b  = mean_n node_feature[b, n, :]          # (D,)
    h1_b = relu(m_b @ W0 + b0)
    h2_b = relu(h1_b @ W1 + b1)
    h3_b = h2_b @ W2 + b2
    out[b, n, :] = node_feature[b, n, :] + h3_b  # broadcast residual

Sharding: data-parallel over batch, 4 batches per core on 8 cores.

Per-core dataflow (all HW-tuned via A/B benching on the axon trn2 pool):
- Loads are split in halves across BOTH HWDGE rings (SP + ACT) so the two
  rings run concurrently; stores go out via the SWDGE (gpsimd) path (plus
  HWDGE halves), keeping every DMA queue single-direction — mixing
  directions on one queue causes head-of-line blocking at the sequencer.
- Per-batch column sums run on the PE (data as stationary, ones vector
  moving, PSUM accumulation), the 256x256 chain runs in transposed
  orientation (weights as stationary, h as a 1-column moving operand),
  bias+relu is a single DVE tensor_scalar op, the h3 broadcast across
  partitions is a rank-1 PE matmul, and the residual add is an in-place
  DVE tensor_tensor. No compute ever lands on the DMA-issuing engines
  (SP/ACT/gpsimd), which benchmarks showed starves the DMA queues.
"""

import numpy as np

import concourse.bacc as bacc
import concourse.bass as bass
import concourse.mybir as mybir
import concourse.tile as tile
from concourse.bass_utils import run_bass_kernel_spmd

F32 = mybir.dt.float32

B, N, D, L = 32, 1000, 256, 3
NCORES = 8
NB = B // NCORES  # batches per core
P = 125           # partition rows per node-slice
T = N // P        # node-slices per batch
HALF = 128        # half of D (partition dim for transposed chain)

# DMA queue assignment (per batch): loads split in halves across the two
# HWDGE rings; stores mostly on the SWDGE (gpsimd) queue.
LOAD_ENGINES = [["sync", "scalar"]] * NB
STORE_ENGINES = [["gpsimd"]] * NB

_NC_CACHE = {}


def _build_nc(reps=1):
    nc = bacc.Bacc("TRN2", target_bir_lowering=False, debug=False)

    nf_d = nc.dram_tensor("nf", [NB, N, D], F32, kind="ExternalInput")
    w_d = nc.dram_tensor("w", [L, D, D], F32, kind="ExternalInput")
    bvec_d = nc.dram_tensor("bvec", [HALF, 2 * L], F32, kind="ExternalInput")
    out_d = nc.dram_tensor("out", [NB, N, D], F32, kind="ExternalOutput")

    ones_col_d = nc.inline_tensor(np.ones((P, 1), np.float32), "ones_col")
    ones_row_d = nc.inline_tensor(np.ones((1, P), np.float32), "ones_row")
    ident_d = nc.inline_tensor(np.eye(HALF, dtype=np.float32), "ident")

    add_op = mybir.AluOpType.add
    max_op = mybir.AluOpType.max

    with tile.TileContext(nc) as tc:
        with (
            tc.tile_pool(name="const", bufs=1) as cpool,
            tc.tile_pool(name="data", bufs=NB) as dpool,
            tc.tile_pool(name="vec", bufs=8) as vpool,
            tc.tile_pool(name="bcast", bufs=2) as bpool,
            tc.tile_pool(name="ps_sum", bufs=2, space=bass.MemorySpace.PSUM) as ps_sum,
            tc.tile_pool(name="ps_chain", bufs=2, space=bass.MemorySpace.PSUM) as ps_chain,
            tc.tile_pool(name="ps_row", bufs=2, space=bass.MemorySpace.PSUM) as ps_row,
            tc.tile_pool(name="ps_bc", bufs=2, space=bass.MemorySpace.PSUM) as ps_bc,
        ):
            # ---- constants ----
            w_sb = []
            for l in range(L):
                wt = cpool.tile([HALF, 2, D], F32, tag=f"w{l}", name=f"w{l}")
                nc.sync.dma_start(wt[:], w_d[l].rearrange("(kc k) e -> k kc e", k=HALF))
                w_sb.append(wt)
            bvec = cpool.tile([HALF, 2 * L], F32, tag="bvec", name="bvec")
            nc.sync.dma_start(bvec[:], bvec_d[:])
            ones_col = cpool.tile([P, 1], F32, tag="ones_col", name="ones_col")
            nc.sync.dma_start(ones_col[:], ones_col_d[:])
            ones_row = cpool.tile([1, P], F32, tag="ones_row", name="ones_row")
            nc.sync.dma_start(ones_row[:], ones_row_d[:])
            ident = cpool.tile([HALF, HALF], F32, tag="ident", name="ident")
            nc.sync.dma_start(ident[:], ident_d[:])

            def batch_body():
                for b in range(NB):
                    nf_t = dpool.tile([P, T, D], F32, tag="nf", name=f"nf{b}")
                    src = nf_d[b].rearrange("(t p) d -> p t d", p=P)
                    spec = LOAD_ENGINES[b]
                    step = T // len(spec)
                    for s, eng in enumerate(spec):
                        getattr(nc, eng).dma_start(
                            nf_t[:, s * step:(s + 1) * step, :],
                            src[:, s * step:(s + 1) * step, :],
                        )

                    # per-batch column sums (transposed orientation):
                    # sumT[mh][d, 0] = sum_n nf[b, n, mh*128 + d]
                    h = []
                    for mh in range(2):
                        ps = ps_sum.tile([HALF, 1], F32, tag="ps_s", name=f"ps_s{b}_{mh}")
                        for t in range(T):
                            nc.tensor.matmul(
                                ps[:],
                                nf_t[:, t, mh * HALF:(mh + 1) * HALF],
                                ones_col[:],
                                start=(t == 0),
                                stop=(t == T - 1),
                            )
                        s = vpool.tile([HALF, 1], F32, tag="hT", name=f"sum{b}_{mh}")
                        nc.vector.tensor_scalar_mul(s[:], ps[:], 1.0 / N)
                        h.append(s)

                    # 3-layer chain, transposed orientation, bias+relu on DVE
                    for l in range(L):
                        hn = []
                        for mh in range(2):
                            pc = ps_chain.tile(
                                [HALF, 1], F32, tag="ps_c", name=f"ps_c{b}_{l}_{mh}"
                            )
                            for kc in range(2):
                                nc.tensor.matmul(
                                    pc[:],
                                    w_sb[l][:, kc, mh * HALF:(mh + 1) * HALF],
                                    h[kc][:],
                                    start=(kc == 0),
                                    stop=(kc == 1),
                                )
                            ht = vpool.tile([HALF, 1], F32, tag="hT", name=f"h{b}_{l}_{mh}")
                            bias_ap = bvec[:, 2 * l + mh:2 * l + mh + 1]
                            if l < L - 1:
                                nc.vector.tensor_scalar(
                                    ht[:], pc[:], bias_ap, 0.0, add_op, max_op
                                )
                            else:
                                nc.vector.tensor_scalar_add(ht[:], pc[:], bias_ap)
                            hn.append(ht)
                        h = hn

                    # transpose h3 back to a row, broadcast across partitions
                    pr = ps_row.tile([1, D], F32, tag="ps_r", name=f"ps_r{b}")
                    for kc in range(2):
                        nc.tensor.transpose(
                            pr[0:1, kc * HALF:(kc + 1) * HALF], h[kc][:], ident[:]
                        )
                    h3row = vpool.tile([1, D], F32, tag="h3row", name=f"h3row{b}")
                    nc.vector.tensor_copy(h3row[:], pr[:])
                    pb = ps_bc.tile([P, D], F32, tag="ps_b", name=f"ps_b{b}")
                    nc.tensor.matmul(pb[:], ones_row[:], h3row[:], start=True, stop=True)
                    bc = bpool.tile([P, D], F32, tag="bc", name=f"bc{b}")
                    nc.vector.tensor_copy(bc[:], pb[:])

                    # residual add in place, then stream out
                    for t in range(T):
                        nc.vector.tensor_add(nf_t[:, t, :], nf_t[:, t, :], bc[:])
                    dst = out_d[b].rearrange("(t p) d -> p t d", p=P)
                    spec = STORE_ENGINES[b]
                    step = T // len(spec)
                    for s, eng in enumerate(spec):
                        getattr(nc, eng).dma_start(
                            dst[:, s * step:(s + 1) * step, :],
                            nf_t[:, s * step:(s + 1) * step, :],
                        )

            if reps == 1:
                batch_body()
            else:
                with tc.For_i(0, reps, 1):
                    batch_body()

    nc.compile()
    return nc


def _get_nc(reps=1):
    if reps not in _NC_CACHE:
        _NC_CACHE[reps] = _build_nc(reps)
    return _NC_CACHE[reps]


def _make_in_maps(node_feature, Ws, bs):
    nf = np.ascontiguousarray(np.asarray(node_feature, dtype=np.float32))
    w = np.ascontiguousarray(np.asarray(Ws, dtype=np.float32))
    b = np.asarray(bs, dtype=np.float32)
    # bvec[p, 2*l + half] = bs[l, half*128 + p]
    bvec = np.ascontiguousarray(
        b.reshape(L, 2, HALF).transpose(2, 0, 1).reshape(HALF, 2 * L)
    )
    in_maps = []
    for i in range(NCORES):
        in_maps.append(
            {
                "nf": np.ascontiguousarray(nf[i * NB:(i + 1) * NB]),
                "w": w,
                "bvec": bvec,
            }
        )
    return in_maps


def run_on_hw(node_feature, Ws, bs):
    nc = _get_nc()
    res = run_bass_kernel_spmd(
        nc,
        _make_in_maps(node_feature, Ws, bs),
        list(range(NCORES)),
        trace=False,
    )
    out = np.concatenate([res.results[i]["out"] for i in range(NCORES)], axis=0)
    return out, res


def kernel(x, node_feature, Ws, bs):
    node_feature = np.asarray(node_feature, dtype=np.float32)
    out, _ = run_on_hw(node_feature, Ws, bs)
    return out, node_feature


# ---------------------------------------------------------------------------
# Timing runner: same PJRT path as run_bass_kernel_spmd under axon, but with
# the jitted executable cached so repeated executions can be timed without
# re-tracing/re-compiling. Used by test.py only.
# ---------------------------------------------------------------------------


class _Runner:
    def __init__(self, nc=None):
        import jax
        from jax.experimental.shard_map import shard_map
        from jax.sharding import Mesh, NamedSharding, PartitionSpec

        from concourse.bass2jax import (
            _bass_exec_p,
            install_neuronx_cc_hook,
            partition_id_tensor,
        )

        install_neuronx_cc_hook()
        self.jax = jax
        if nc is None:
            nc = _get_nc(1)
        partition_name = (
            nc.partition_id_tensor.name if nc.partition_id_tensor else None
        )
        in_names, out_names, out_avals, zero_outs = [], [], [], []
        for alloc in nc.m.functions[0].allocations:
            if not isinstance(alloc, mybir.MemoryLocationSet):
                continue
            name = alloc.memorylocations[0].name
            if alloc.kind == "ExternalInput":
                if name != partition_name:
                    in_names.append(name)
            elif alloc.kind == "ExternalOutput":
                shape = tuple(alloc.tensor_shape)
                dt = mybir.dt.np(alloc.dtype)
                out_names.append(name)
                out_avals.append(jax.core.ShapedArray(shape, dt))
                zero_outs.append(np.zeros(shape, dt))
        self.in_names = in_names
        self.out_names = out_names
        self.out_avals = out_avals
        self.zero_outs = zero_outs
        n_params, n_outs = len(in_names), len(out_names)
        all_names = tuple(
            in_names + out_names + ([partition_name] if partition_name else [])
        )

        def _body(*args):
            operands = list(args)
            if partition_name is not None:
                operands.append(partition_id_tensor())
            outs = _bass_exec_p.bind(
                *operands,
                out_avals=tuple(out_avals),
                in_names=all_names,
                out_names=tuple(out_names),
                lowering_input_output_aliases=(),
                sim_require_finite=True,
                sim_require_nnan=True,
                nc=nc,
            )
            return tuple(outs)

        devices = jax.devices()[:NCORES]
        self.mesh = Mesh(np.asarray(devices), ("core",))
        self.sharding = NamedSharding(self.mesh, PartitionSpec("core"))
        in_specs = (PartitionSpec("core"),) * (n_params + n_outs)
        out_specs = (PartitionSpec("core"),) * n_outs
        self.jitted = jax.jit(
            shard_map(
                _body,
                mesh=self.mesh,
                in_specs=in_specs,
                out_specs=out_specs,
                check_rep=False,
            ),
            donate_argnums=tuple(range(n_params, n_params + n_outs)),
            keep_unused=True,
        )

    def stage_inputs(self, in_maps):
        concat = [
            np.concatenate([m[name] for m in in_maps], axis=0)
            for name in self.in_names
        ]
        return [self.jax.device_put(a, self.sharding) for a in concat]

    def stage_zeros(self):
        return [
            self.jax.device_put(
                np.zeros((NCORES * z.shape[0], *z.shape[1:]), z.dtype), self.sharding
            )
            for z in self.zero_outs
        ]

    def run(self, dev_inputs, dev_zeros):
        return self.jitted(*dev_inputs, *dev_zeros)


_RUNNER_CACHE = {}


def get_runner(reps=1):
    if reps not in _RUNNER_CACHE:
        _RUNNER_CACHE[reps] = _Runner(_get_nc(reps))
    return _RUNNER_CACHE[reps]
